# revision 2
# baseline (speedup 1.0000x reference)
"""Bidirectional GRU (Keras v2, reset_after=True, masked) on 8 Trainium2 cores.

Sharding: direction-split x batch-split. Cores 0-3 run the forward GRU on
batch slices of 16; cores 4-7 run the backward GRU on time-reversed inputs
(so all 8 cores execute the identical SPMD program). Weights are replicated
within a direction group.

Per-core kernel, two phases:
  1. Projection: xg = x @ W for all timesteps, tiled on the PE with W-tiles
     stationary, producing xg in transposed layout (gate-units on partitions)
     directly in SBUF as bf16. The contiguous-padding mask is folded in as a
     +30000 bias on the z-gate columns (sigmoid(big) == 1.0 makes the state
     carry through masked steps exactly), via an extra K=1 matmul.
  2. Scan: 250 sequential steps. hg^T = U^T h with U-tiles stationary (bf16
     -> fast weight load) accumulating over 4 K-chunks into PSUM; gate math
     in fp32 with units on partitions: sigmoid/tanh on ScalarE, elementwise
     on VectorE. The 512 hidden units are processed as two granules of 256
     so gate math of one granule overlaps PE work of the other.

Column blocks of the 3*U=1536 gate dim are permuted to
[z0 z1 r0 r1 h0 h1 | z2 z3 r2 r3 h2 h3] (128 cols each) so each granule's
z/r block is contiguous (one sigmoid op) and h' chunks line up with the
K-chunks of the next step's matmuls.
"""

import numpy as np
import ml_dtypes

import concourse.bass as bass
import concourse.tile as tile
from concourse import mybir
from concourse.bass_utils import run_bass_kernel_spmd

# ---------------------------------------------------------------------------
# Workaround: this walrus build rejects instructions carrying more than one
# semaphore wait ("Too many sync wait commands", setupSyncWait<...>). After
# Tile's wait-assignment pass, hoist excess waits of every instruction onto
# single-wait nops on the same engine, placed immediately before it (same
# semantics: the engine blocks on each wait in order).
import bass_rust
from concourse.vector_clock import ScopedClock

_MAX_CTRL_WAITS = 1
_WSPLIT = [0]

_orig_lower_ordered = tile.TileContext._lower_ordered_insts


def _patched_lower_ordered(self, ordered):
    for _bb, insts in ordered.items():
        out = []
        for inst in insts:
            si = getattr(inst, "sync_info", None)
            ow = getattr(si, "on_wait", None) if si is not None else None
            if ow is not None and len(ow) > _MAX_CTRL_WAITS:
                extra = list(ow[_MAX_CTRL_WAITS:])
                del ow[_MAX_CTRL_WAITS:]
                for w in extra:
                    _WSPLIT[0] += 1
                    nop = bass_rust.InstNoOp(
                        name=f"I-wsplit-{_WSPLIT[0]}",
                        ins=[],
                        outs=[],
                        engine=inst.engine,
                        text_hint="wsplit",
                    )
                    nop.sync_info = mybir.SyncInfo(on_wait=[w], on_update=[])
                    out.append(nop)
            out.append(inst)
        insts[:] = out
    return _orig_lower_ordered(self, ordered)


tile.TileContext._lower_ordered_insts = _patched_lower_ordered


def _patched_drain_and_barrier(self, tick_clock, wait_clock):
    drain_inst = self.nc.sync.drain()
    wait_clock.add_sem_waits(
        drain_inst.ins, ScopedClock({None: tick_clock.global_clock})
    )
    ow = drain_inst.ins.sync_info.on_wait
    if len(ow) > _MAX_CTRL_WAITS:
        extra = list(ow[_MAX_CTRL_WAITS:])
        del ow[_MAX_CTRL_WAITS:]
        for i in range(0, len(extra), _MAX_CTRL_WAITS):
            nop = self.nc.sync.nop(nofuse=True, hint="drain_wait_split")
            if nop.ins.sync_info is None:
                nop.ins.sync_info = mybir.SyncInfo(on_wait=[], on_update=[])
            nop.ins.sync_info.on_wait.extend(extra[i : i + _MAX_CTRL_WAITS])

    self.nc.all_engine_barrier()
    assert self.sems is not None
    popped = self.nc._tile_sem_poison_stack.pop()
    assert popped is self._sem_poison
    self.nc.clear_and_free_semaphores(list(self.sems.allocated().values()))
    self.nc.all_engine_barrier()


tile.TileContext._drain_and_barrier = _patched_drain_and_barrier
# ---------------------------------------------------------------------------

BF16 = mybir.dt.bfloat16
F32 = mybir.dt.float32
NP_BF16 = ml_dtypes.bfloat16

B, T, C, U = 64, 250, 1024, 512
NCORES = 8
BL = 16          # batch per core
G = 3 * U        # 1536
NBLK = 12        # 128-col blocks of G
TCH = 25         # projection time-chunk -> N = 25*16 = 400 <= 512
NCH = T // TCH   # 10
MASK_BIG = 30000.0

# natural block index (z_u=u, r_u=4+u, h_u=8+u) in permuted slot order:
# granule 0: z0 z1 r0 r1 h0 h1 ; granule 1: z2 z3 r2 r3 h2 h3
PERM = [0, 1, 4, 5, 8, 9, 2, 3, 6, 7, 10, 11]
PERM_COLS = np.concatenate([np.arange(128) + 128 * b for b in PERM])
Z_SLOTS = (0, 1, 6, 7)  # permuted slots holding z-gate blocks

SIG = mybir.ActivationFunctionType.Sigmoid
TANH = mybir.ActivationFunctionType.Tanh


def build_nc():
    nc = bass.Bass()
    xT = nc.dram_tensor("xT", [C, T, BL], BF16, kind="ExternalInput")
    Wp = nc.dram_tensor("Wp", [C, G], BF16, kind="ExternalInput")
    Up = nc.dram_tensor("Up", [U, G], BF16, kind="ExternalInput")
    mrow = nc.dram_tensor("mrow", [1, T * BL], BF16, kind="ExternalInput")
    y = nc.dram_tensor("y", [T, 2, 128, 2 * BL], F32, kind="ExternalOutput")
    hT = nc.dram_tensor("hT", [2, 128, 2 * BL], F32, kind="ExternalOutput")

    with tile.TileContext(nc) as tc:
        with (
            tc.tile_pool(name="const", bufs=1) as cpool,
            tc.tile_pool(name="xin", bufs=2) as xpool,
            tc.tile_pool(name="ppsum", bufs=3, space="PSUM") as ppsum,
            tc.tile_pool(name="spsum", bufs=2, space="PSUM") as spsum,
            tc.tile_pool(name="work", bufs=2) as work,
            tc.tile_pool(name="state", bufs=3) as state,
        ):
            # ---- resident tensors ----
            W_sb = cpool.tile([128, C // 128, G], BF16, tag="W")
            for k in range(C // 128):
                nc.sync.dma_start(W_sb[:, k, :], Wp[k * 128 : (k + 1) * 128, :])
            U_sb = cpool.tile([128, U // 128, G], BF16, tag="U")
            for k in range(U // 128):
                nc.sync.dma_start(U_sb[:, k, :], Up[k * 128 : (k + 1) * 128, :])
            m_sb = cpool.tile([1, T * BL], BF16, tag="m")
            nc.sync.dma_start(m_sb[:], mrow[:])
            ones = cpool.tile([1, 128], BF16, tag="ones")
            nc.gpsimd.memset(ones[:], 1.0)
            xg = cpool.tile([128, T, NBLK * BL], BF16, tag="xg")

            # ---- phase 1: xg = x @ W (+ mask bias on z slots) ----
            for ci in range(NCH):
                t0 = ci * TCH
                xts = []
                for k in range(C // 128):
                    xt = xpool.tile([128, TCH, BL], BF16, tag=f"x{k}")
                    nc.sync.dma_start(
                        xt[:], xT[k * 128 : (k + 1) * 128, t0 : t0 + TCH, :]
                    )
                    xts.append(xt)
                for s in range(NBLK):
                    zmm = s in Z_SLOTS
                    ps = ppsum.tile([128, TCH, BL], F32, tag="pp")
                    for k in range(C // 128):
                        nc.tensor.matmul(
                            ps[:],
                            W_sb[:, k, s * 128 : (s + 1) * 128],
                            xts[k][:],
                            start=(k == 0),
                            stop=(k == C // 128 - 1 and not zmm),
                        )
                    if zmm:
                        nc.tensor.matmul(
                            ps[:],
                            ones[:],
                            m_sb[:, t0 * BL : (t0 + TCH) * BL],
                            start=False,
                            stop=True,
                        )
                    nc.vector.tensor_copy(
                        xg[:, t0 : t0 + TCH, s * BL : (s + 1) * BL], ps[:]
                    )

            # ---- phase 2: sequential scan ----
            h_prev = []
            hc_prev = []
            for g in range(2):
                h0 = state.tile([128, 2 * BL], F32, tag=f"h{g}")
                nc.gpsimd.memset(h0[:], 0.0)
                hc0 = state.tile([128, 2 * BL], BF16, tag=f"hc{g}")
                nc.gpsimd.memset(hc0[:], 0.0)
                h_prev.append(h0)
                hc_prev.append(hc0)

            for t in range(T):
                h_new = [None, None]
                hc_new = [None, None]
                for g in range(2):
                    ps = spsum.tile([128, 6 * BL], F32, tag=f"sp{g}")
                    # hg^T for this granule: 6 slots x 4 K-chunks
                    for j in range(6):
                        s = g * 6 + j
                        for k in range(4):
                            nc.tensor.matmul(
                                ps[:, j * BL : (j + 1) * BL],
                                U_sb[:, k, s * 128 : (s + 1) * 128],
                                hc_prev[k // 2][:, (k % 2) * BL : (k % 2 + 1) * BL],
                                start=(k == 0),
                                stop=(k == 3),
                            )
                    xbase = g * 6 * BL
                    # z,r: sigmoid(x-part + h-part)  (one op for both gates)
                    sin = work.tile([128, 4 * BL], F32, tag=f"sin{g}")
                    nc.vector.tensor_add(
                        sin[:], ps[:, 0 : 4 * BL], xg[:, t, xbase : xbase + 4 * BL]
                    )
                    zr = work.tile([128, 4 * BL], F32, tag=f"zr{g}")
                    nc.scalar.activation(zr[:], sin[:], SIG)
                    # c = tanh(xh + r * hh)
                    t1 = work.tile([128, 2 * BL], F32, tag=f"t1{g}")
                    nc.vector.tensor_mul(
                        t1[:], zr[:, 2 * BL : 4 * BL], ps[:, 4 * BL : 6 * BL]
                    )
                    t2 = work.tile([128, 2 * BL], F32, tag=f"t2{g}")
                    nc.vector.tensor_add(
                        t2[:], t1[:], xg[:, t, xbase + 4 * BL : xbase + 6 * BL]
                    )
                    c = work.tile([128, 2 * BL], F32, tag=f"c{g}")
                    nc.scalar.activation(c[:], t2[:], TANH)
                    # h' = c + z*(h - c)
                    d = work.tile([128, 2 * BL], F32, tag=f"d{g}")
                    nc.vector.tensor_sub(d[:], h_prev[g][:], c[:])
                    e = work.tile([128, 2 * BL], F32, tag=f"e{g}")
                    nc.vector.tensor_mul(e[:], zr[:, 0 : 2 * BL], d[:])
                    hn = state.tile([128, 2 * BL], F32, tag=f"h{g}")
                    nc.vector.tensor_add(hn[:], c[:], e[:])
                    hc = state.tile([128, 2 * BL], BF16, tag=f"hc{g}")
                    nc.vector.tensor_copy(hc[:], hn[:])
                    nc.sync.dma_start(y[t, g], hn[:])
                    h_new[g] = hn
                    hc_new[g] = hc
                h_prev = h_new
                hc_prev = hc_new

            for g in range(2):
                nc.sync.dma_start(hT[g], h_prev[g][:])

    return nc


_NC_CACHE = None


def _get_nc():
    global _NC_CACHE
    if _NC_CACHE is None:
        _NC_CACHE = build_nc()
    return _NC_CACHE


def make_in_maps(inputs, mask, Wf, Uf, bf, Wb, Ub, bb):
    assert not np.any(bf) and not np.any(bb), "nonzero GRU biases unsupported"
    in_maps = []
    for core in range(NCORES):
        d = core // 4
        b0 = (core % 4) * BL
        x = np.asarray(inputs[b0 : b0 + BL], dtype=np.float32)
        m = np.asarray(mask[b0 : b0 + BL])
        W, Uk = (Wf, Uf) if d == 0 else (Wb, Ub)
        if d == 1:
            x = x[:, ::-1]
            m = m[:, ::-1]
        xT = np.ascontiguousarray(x.transpose(2, 1, 0)).astype(NP_BF16)
        mrow = (
            ((1.0 - m.T.astype(np.float32)) * MASK_BIG)
            .reshape(1, -1)
            .astype(NP_BF16)
        )
        Wp = np.asarray(W, dtype=np.float32)[:, PERM_COLS].astype(NP_BF16)
        Up = np.asarray(Uk, dtype=np.float32)[:, PERM_COLS].astype(NP_BF16)
        in_maps.append({"xT": xT, "Wp": Wp, "Up": Up, "mrow": mrow})
    return in_maps


def assemble(results):
    """results: list of 8 dicts with 'y' [T,2,128,32] and 'hT' [2,128,32]."""
    ys = []
    hs = []
    for core in range(NCORES):
        y = np.asarray(results[core]["y"], dtype=np.float32)
        h = np.asarray(results[core]["hT"], dtype=np.float32)
        # y[t, g, p, ul*16+b] -> [b, t, unit = (2g+ul)*128+p]
        yb = y.reshape(T, 2, 128, 2, BL).transpose(4, 0, 1, 3, 2).reshape(BL, T, U)
        hb = h.reshape(2, 128, 2, BL).transpose(3, 0, 2, 1).reshape(BL, U)
        if core // 4 == 1:
            yb = yb[:, ::-1]
        ys.append(yb)
        hs.append(hb)
    y_f = np.concatenate(ys[0:4], axis=0)
    y_b = np.concatenate(ys[4:8], axis=0)
    h_f = np.concatenate(hs[0:4], axis=0)
    h_b = np.concatenate(hs[4:8], axis=0)
    seq_out = np.concatenate([y_f, y_b], axis=-1)
    state_out = np.concatenate([h_f, h_b], axis=-1)
    return seq_out, state_out


def kernel(inputs, mask, Wf, Uf, bf, Wb, Ub, bb):
    nc = _get_nc()
    in_maps = make_in_maps(inputs, mask, Wf, Uf, bf, Wb, Ub, bb)
    res = run_bass_kernel_spmd(nc, in_maps, core_ids=list(range(NCORES)))
    return assemble(res.results)
